# revision 1
# baseline (speedup 1.0000x reference)
"""DeepSeekMoE (top-2 of 8 experts + 2x shared expert) on 8 Trainium2 NeuronCores.

Strategy (hardcoded for x:[4,2048,2048], D=2048, H=1024, E=8, K=2):
  - Host: gating (logits/top-2/softmax) in float64 numpy; expert-parallel
    dispatch -- core e receives the tokens routed to expert e, gathered,
    transposed to [D, C] and padded to a common capacity C.  Shared-expert
    work is data-parallel: core c gets tokens [1024c, 1024(c+1)).
  - Device (SPMD, same program on all 8 cores): two back-to-back FFN
    pipelines computed fully transposed (outputs [D, tokens]) so that both
    layers contract over the partition dim with natural-layout weights:
      hT = gelu(w1.T @ xT + b1); yrT = w2.T @ hT        (routed, C tokens)
      hsT = gelu(sw1.T @ xsT + sb1); ysT = sw2.T @ hsT  (shared, 1024 tokens)
    b2/sb2 biases and the top-2 gate scaling are folded into the host-side
    combine (they are affine post-matmul terms).
  - Host: out[tok] += gate * (yr + b2[e]) scatter per expert; += ys + sb2.

All device tensors use per-tile-blocked DRAM layouts (per-partition
contiguous >=2KB lines) -- measured ~1.6x DMA bandwidth vs strided views.
Matmul dtype strategy selectable; fp16 operands with fp32 PSUM accumulation
give ~6e-6 end-to-end absmax relative error vs a float64 reference.
"""
import contextlib
import os
import sys
import numpy as np

for _p in ("/root/.axon_site/_ro/trn_rl_repo", "/root/.axon_site/_ro/pypackages",
           "/opt/trn_rl_repo", "/opt/pypackages"):
    if os.path.isdir(_p) and _p not in sys.path:
        sys.path.append(_p)

import ml_dtypes
from concourse import bacc, mybir
from concourse import tile
from concourse.bass_utils import run_bass_kernel_spmd

BF16 = ml_dtypes.bfloat16

# ---- problem constants (hardcoded per spec) ----
B, S, D, H, E = 4, 2048, 2048, 1024, 8
SH = 2 * H
N = B * S                    # 8192 tokens
TOPK = 2
NCORES = 8
TS = N // NCORES             # shared-expert tokens per core (1024)
ND = D // 128                # 16 contraction chunks over D
NH = H // 128                # 8 chunks over H
NSH = SH // 128              # 16 chunks over SH
TILE_N = 512                 # token-tile width (one PSUM bank of fp32)
NTS = TS // TILE_N           # shared-expert token tiles per core (2)

STRATEGY = os.environ.get("MOE_STRATEGY", "f16")   # f16 | bf16 | f32

_DT = {
    "f16": (mybir.dt.float16, np.float16),
    "bf16": (mybir.dt.bfloat16, BF16),
    "f32": (mybir.dt.float32, np.float32),
}

F32 = mybir.dt.float32

LAST_RESULTS = None          # BassKernelResults of the most recent device run

_BUILD_CACHE = {}


def _ntiles(total):
    """(offset, width) token tiles covering `total` (512s, then remainder)."""
    out = []
    t = 0
    while t < total:
        w = min(TILE_N, total - t)
        out.append((t, w))
        t += w
    return out


def _build(C, strategy, loop_iters=None):
    """Build + compile the per-core SPMD program for capacity C.

    loop_iters: if set, wrap the whole body in a device-side For_i that
    repeats it that many times (benchmarking only -- the body is idempotent).
    """
    # bench-only experiment knobs (wrong numerics when set -- never for real runs)
    act_copy = os.environ.get("MOE_ACT_COPY") == "1"
    no_store = os.environ.get("MOE_NO_STORE") == "1"
    ps_bufs = int(os.environ.get("MOE_PS_BUFS", "4"))
    store_eng = os.environ.get("MOE_STORE_ENG", "dve")   # dve|act|alt
    dup_r = int(os.environ.get("MOE_DUP_R", "1"))        # emit routed phase N times (bench)
    dup_s = int(os.environ.get("MOE_DUP_S", "1"))        # emit shared phase N times (bench)
    key = (C, strategy, loop_iters, act_copy, no_store, ps_bufs, store_eng,
           dup_r, dup_s)
    if key in _BUILD_CACHE:
        return _BUILD_CACHE[key]
    dm, _ = _DT[strategy]
    stream_weights = strategy == "f32"   # 4-byte weights don't fit resident

    nc = bacc.Bacc("TRN2", target_bir_lowering=False, debug=False)

    # Weights: per-column-tile layout [ncols, 128(p), kchunks*128], element
    # (col, p, kc*128+c) = w[kc*128 + p, col*128 + c].
    # x: flat, blocked per token tile: block(n) is [128(p), ND*nw] with
    # element (p, d*nw+c) = xT2d[d*128 + p, n0 + c].
    # y: flat, blocked per (token tile, dt): tile block [128(p), nw] =
    # yT2d[dt*128 + p, n0:n0+nw]; for one n the ND blocks are consecutive.
    d_xT = nc.dram_tensor("xT", [D * C], dm, kind="ExternalInput")
    d_w1 = nc.dram_tensor("w1", [NH, 128, ND * 128], dm, kind="ExternalInput")
    d_w2 = nc.dram_tensor("w2", [ND, 128, NH * 128], dm, kind="ExternalInput")
    d_b1 = nc.dram_tensor("b1c", [H, 1], F32, kind="ExternalInput")
    d_xsT = nc.dram_tensor("xsT", [NTS, 128, ND * TILE_N], dm, kind="ExternalInput")
    d_sw1 = nc.dram_tensor("sw1", [NSH, 128, ND * 128], dm, kind="ExternalInput")
    d_sw2 = nc.dram_tensor("sw2", [ND, 128, NSH * 128], dm, kind="ExternalInput")
    d_sb1 = nc.dram_tensor("sb1c", [SH, 1], F32, kind="ExternalInput")
    d_yrT = nc.dram_tensor("yrT", [D * C], F32, kind="ExternalOutput")
    d_ysT = nc.dram_tensor("ysT", [NTS, ND, 128, TILE_N], F32, kind="ExternalOutput")

    v_b1 = d_b1.ap().rearrange("(b p) o -> p b o", p=128)     # [128, NH, 1]
    v_sb1 = d_sb1.ap().rearrange("(b p) o -> p b o", p=128)   # [128, NSH, 1]

    def x_tile_view(n0, nw):
        a = d_xT.ap()[D * n0: D * (n0 + nw)]
        return a.rearrange("(p d c) -> p d c", p=128, d=ND)

    def xs_tile_view(n):
        return d_xsT.ap()[n].rearrange("p (d c) -> p d c", d=ND)

    def yr_tile_view(n0, nw, dt):
        a = d_yrT.ap()[D * n0 + dt * 128 * nw: D * n0 + (dt + 1) * 128 * nw]
        return a.rearrange("(p c) -> p c", p=128)

    def wcol_view(d_w, col, kchunks):
        return d_w.ap()[col].rearrange("p (k c) -> p k c", k=kchunks)

    gelu = mybir.ActivationFunctionType.Gelu
    rtiles = _ntiles(C)

    with tile.TileContext(nc) as tc:
        with tc.tile_pool(name="wres", bufs=1) as wres, \
             tc.tile_pool(name="wcol", bufs=3) as wcol, \
             tc.tile_pool(name="xs", bufs=3) as xs, \
             tc.tile_pool(name="hp", bufs=2) as hp, \
             tc.tile_pool(name="bias", bufs=1) as bias, \
             tc.tile_pool(name="ps1", bufs=ps_bufs, space="PSUM") as ps1, \
             tc.tile_pool(name="ps2", bufs=ps_bufs, space="PSUM") as ps2, \
             tc.tile_pool(name="st", bufs=4) as stp:
          with (tc.For_i(0, loop_iters, 1) if loop_iters is not None
                else contextlib.nullcontext()):

            # biases first (tiny; the DMA FIFO runs in emission order and the
            # first gelu needs b1), then the first token tile.
            b1t = bias.tile([128, NH], F32, tag="b1")
            sb1t = bias.tile([128, NSH], F32, tag="sb1")
            nc.sync.dma_start(out=b1t[:], in_=v_b1[:, :, 0])
            nc.sync.dma_start(out=sb1t[:], in_=v_sb1[:, :, 0])

            # warm the ACT gelu table while DMAs stream: the auto-inserted
            # LoadActFuncSet binds to the first Activation in program order.
            warm = bias.tile([128, 1], F32, tag="warm")
            nc.vector.memset(warm[:], 0.0)
            nc.scalar.activation(warm[:], warm[:],
                                 mybir.ActivationFunctionType.Gelu)

            xt0 = xs.tile([128, ND, TILE_N], dm, tag="xstream")
            nc.sync.dma_start(out=xt0[:, :, :rtiles[0][1]],
                              in_=x_tile_view(*rtiles[0]))

            # routed weights: resident per-column tiles (fine-grained deps
            # so the first matmul only waits on its own 0.25 MB column)
            if not stream_weights:
                w1cols = []
                for ht in range(NH):
                    t = wres.tile([128, ND, 128], dm, tag=f"w1c{ht}")
                    nc.sync.dma_start(out=t[:], in_=wcol_view(d_w1, ht, ND))
                    w1cols.append(t)
                w2cols = []
                for dt in range(ND):
                    t = wres.tile([128, NH, 128], dm, tag=f"w2c{dt}")
                    nc.sync.dma_start(out=t[:], in_=wcol_view(d_w2, dt, NH))
                    w2cols.append(t)

            def w1_col(ht):
                if not stream_weights:
                    return w1cols[ht]
                t = wcol.tile([128, ND, 128], dm, tag="wa")
                nc.sync.dma_start(out=t[:], in_=wcol_view(d_w1, ht, ND))
                return t

            def w2_col(dt):
                if not stream_weights:
                    return w2cols[dt]
                t = wcol.tile([128, NH, 128], dm, tag="wb")
                nc.sync.dma_start(out=t[:], in_=wcol_view(d_w2, dt, NH))
                return t

            _store_ctr = [0]

            def store(dram_ap, psum_ap, nw):
                if no_store:
                    return
                if store_eng == "direct":
                    nc.sync.dma_start(out=dram_ap, in_=psum_ap)
                    return
                ot = stp.tile([128, TILE_N], F32, tag="stage")
                eng = store_eng
                if eng == "alt":
                    eng = "dve" if _store_ctr[0] % 2 == 0 else "act"
                    _store_ctr[0] += 1
                if eng == "act":
                    nc.scalar.activation(ot[:, :nw], psum_ap,
                                         mybir.ActivationFunctionType.Copy)
                else:
                    nc.vector.tensor_copy(ot[:, :nw], psum_ap)
                nc.sync.dma_start(out=dram_ap, in_=ot[:, :nw])

            def act(out_ap, psum_ap, bias_ap):
                if act_copy:
                    nc.vector.tensor_copy(out_ap, psum_ap)
                else:
                    nc.scalar.activation(out_ap, psum_ap, gelu,
                                         bias=bias_ap, scale=1.0)

            # ---- routed phase ----
            for n, (n0, nw) in enumerate(rtiles * dup_r):
                if n == 0:
                    xt = xt0
                else:
                    xt = xs.tile([128, ND, TILE_N], dm, tag="xstream")
                    nc.sync.dma_start(out=xt[:, :, :nw], in_=x_tile_view(n0, nw))
                ht_t = hp.tile([128, NH, TILE_N], dm, tag="h")
                for ht in range(NH):
                    wv = w1_col(ht)
                    pt = ps1.tile([128, TILE_N], F32, tag="p1")
                    for d in range(ND):
                        nc.tensor.matmul(pt[:, :nw], lhsT=wv[:, d, :],
                                         rhs=xt[:, d, :nw],
                                         start=(d == 0), stop=(d == ND - 1))
                    act(ht_t[:, ht, :nw], pt[:, :nw], b1t[:, ht:ht + 1])
                for dt in range(ND):
                    wv = w2_col(dt)
                    pt2 = ps2.tile([128, TILE_N], F32, tag="p2")
                    for h in range(NH):
                        nc.tensor.matmul(pt2[:, :nw], lhsT=wv[:, h, :],
                                         rhs=ht_t[:, h, :nw],
                                         start=(h == 0), stop=(h == NH - 1))
                    store(yr_tile_view(n0, nw, dt), pt2[:, :nw], nw)

            # ---- shared expert phase ----
            # st-/dt-outer loops so each sw column streams exactly once.
            for _sdup in range(dup_s):
                xsts = []
                for n in range(NTS):
                    xst = xs.tile([128, ND, TILE_N], dm, tag="xstream")
                    nc.sync.dma_start(out=xst[:], in_=xs_tile_view(n))
                    xsts.append(xst)
                hsts = []
                for _ in range(NTS):
                    hst = hp.tile([128, NSH, TILE_N], dm, tag="hs")
                    hsts.append(hst)
                for st in range(NSH):
                    swv = wcol.tile([128, ND, 128], dm, tag="swa")
                    nc.sync.dma_start(out=swv[:], in_=wcol_view(d_sw1, st, ND))
                    for n in range(NTS):
                        pt = ps1.tile([128, TILE_N], F32, tag="p1")
                        for d in range(ND):
                            nc.tensor.matmul(pt[:], lhsT=swv[:, d, :],
                                             rhs=xsts[n][:, d, :],
                                             start=(d == 0), stop=(d == ND - 1))
                        act(hsts[n][:, st, :], pt[:], sb1t[:, st:st + 1])
                for dt in range(ND):
                    swv2 = wcol.tile([128, NSH, 128], dm, tag="swb")
                    nc.sync.dma_start(out=swv2[:], in_=wcol_view(d_sw2, dt, NSH))
                    for n in range(NTS):
                        pt2 = ps2.tile([128, TILE_N], F32, tag="p2")
                        for sc in range(NSH):
                            nc.tensor.matmul(pt2[:], lhsT=swv2[:, sc, :],
                                             rhs=hsts[n][:, sc, :],
                                             start=(sc == 0), stop=(sc == NSH - 1))
                        store(d_ysT[n, dt, :, :], pt2[:], TILE_N)


    nc.compile()
    _BUILD_CACHE[key] = nc
    return nc


def _route(xf, gate_w):
    """float64 gating: top-2 indices (lax.top_k tie-break) + softmax gates."""
    logits = xf.astype(np.float64) @ np.asarray(gate_w).astype(np.float64)
    order = np.argsort(-logits, axis=1, kind="stable")
    idx = order[:, :TOPK]                                           # [N, 2]
    tl = np.take_along_axis(logits, idx, axis=1)
    tl = tl - tl.max(axis=1, keepdims=True)
    eg = np.exp(tl)
    gates = eg / eg.sum(axis=1, keepdims=True)                      # [N, 2]
    return idx, gates


def _blockT(w, npdt):
    """[K, M] weight -> per-column-tile layout [M/128, 128(p), (K/128)*128]
    with element (col, p, kc*128 + c) = w[kc*128 + p, col*128 + c]."""
    K, M = w.shape
    r = np.asarray(w).astype(npdt).reshape(K // 128, 128, M // 128, 128)
    return np.ascontiguousarray(r.transpose(2, 1, 0, 3)).reshape(M // 128, 128, K)


def _pack_x(xT2d, tiles, npdt):
    """[D, C] -> flat blocked per tile: block(n) [128, ND*nw],
    (p, d*nw+c) = xT2d[d*128+p, n0+c]."""
    r = xT2d.reshape(ND, 128, xT2d.shape[1])
    parts = [np.ascontiguousarray(r[:, :, n0:n0 + nw].transpose(1, 0, 2)).ravel()
             for n0, nw in tiles]
    return np.concatenate(parts).astype(npdt, copy=False)


def _unpack_yr(flat, C):
    """Inverse of the yrT blocked layout -> [D, C] float32."""
    y = np.empty((D, C), np.float32)
    for n0, nw in _ntiles(C):
        y[:, n0:n0 + nw] = flat[D * n0: D * (n0 + nw)].reshape(D, nw)
    return y


def _prepare(x, gate_w, w1, b1, w2, shared_w1, shared_b1, shared_w2, npdt):
    """Host routing + per-core input maps. Returns (C, in_maps, perm, gsel)."""
    xf = np.ascontiguousarray(np.asarray(x).reshape(N, D))
    idx, gates = _route(xf, gate_w)

    perm = []      # token ids routed to each expert (ascending)
    gsel = []      # matching gate weight
    for e in range(E):
        hit0 = idx[:, 0] == e
        hit1 = idx[:, 1] == e
        sel = np.where(hit0 | hit1)[0]
        g = np.where(hit0[sel], gates[sel, 0], gates[sel, 1])
        perm.append(sel)
        gsel.append(g)
    cmax = max(len(p) for p in perm)
    C = ((cmax + 127) // 128) * 128
    rtiles = _ntiles(C)
    stiles = _ntiles(TS)

    xfc = xf.astype(npdt)
    sw1b = _blockT(shared_w1, npdt)
    sw2b = _blockT(shared_w2, npdt)
    sb1c = np.ascontiguousarray(np.asarray(shared_b1).astype(np.float32)).reshape(SH, 1)
    in_maps = []
    for c in range(E):
        sel = perm[c]
        xT = np.zeros((D, C), npdt)
        xT[:, :len(sel)] = xfc[sel].T
        xsT = np.ascontiguousarray(xfc[c * TS:(c + 1) * TS].T)
        in_maps.append({
            "xT": _pack_x(xT, rtiles, npdt),
            "w1": _blockT(w1[c], npdt),
            "w2": _blockT(w2[c], npdt),
            "b1c": np.ascontiguousarray(np.asarray(b1[c]).astype(np.float32)).reshape(H, 1),
            "xsT": _pack_x(xsT, stiles, npdt).reshape(NTS, 128, ND * TILE_N),
            "sw1": sw1b,
            "sw2": sw2b,
            "sb1c": sb1c,
        })
    return C, in_maps, perm, gsel


def kernel(x, gate_w, w1, b1, w2, b2, shared_w1, shared_b1, shared_w2, shared_b2):
    global LAST_RESULTS
    strategy = STRATEGY
    _, npdt = _DT[strategy]

    C, in_maps, perm, gsel = _prepare(
        x, gate_w, w1, b1, w2, shared_w1, shared_b1, shared_w2, npdt)
    nc = _build(C, strategy)

    LAST_RESULTS = run_bass_kernel_spmd(nc, in_maps, core_ids=list(range(NCORES)))
    res = LAST_RESULTS.results

    b2 = np.asarray(b2)
    shared_b2 = np.asarray(shared_b2)
    out = np.zeros((N, D), np.float64)
    for c in range(E):
        sel = perm[c]
        yr = _unpack_yr(res[c]["yrT"], C).T[:len(sel)].astype(np.float64)
        out[sel] += gsel[c][:, None] * (yr + b2[c].astype(np.float64))
        ys = res[c]["ysT"].reshape(NTS, D, TILE_N)                  # [n][D][512]
        ys2d = np.concatenate([ys[n] for n in range(NTS)], axis=1)  # [D, TS]
        out[c * TS:(c + 1) * TS] += ys2d.T.astype(np.float64) + shared_b2.astype(np.float64)

    return out.reshape(B, S, D).astype(np.float32)

